# revision 1
# baseline (speedup 1.0000x reference)
"""Data-parallel kernel for nn_Attention_module_critic_38938173505560.

Shards the batch dim of x (65536) across the 8 NeuronCores (8192 per core),
replicates the tiny parameter set on every core, runs the fused per-shard
attention-critic forward on each core, and gathers the full (65536, 1) output.
No collectives are needed in the forward pass (pure data parallelism).
"""

import jax
import jax.numpy as jnp
import numpy as np

B, N, F_IN, H, D, HID = 65536, 14, 10, 3, 32, 100
N_CORES = 8
B_SHARD = B // N_CORES

WEIGHT_NAMES = [
    "k_w", "k_b", "q_w", "q_b", "v_w", "v_b",
    "kn_g", "kn_b", "qn_g", "qn_b", "vn_g", "vn_b",
    "klin_w", "klin_b", "qlin_w", "qlin_b", "alin_w", "alin_b",
    "l1_w", "l1_b", "l2_w", "l2_b",
    "fc1_w", "fc1_b", "fc2_w", "fc2_b", "fc3_w", "fc3_b",
]


def _ln(x, g, b, n_axes, eps=1e-5):
    ax = tuple(range(x.ndim - n_axes, x.ndim))
    m = jnp.mean(x, ax, keepdims=True)
    v = jnp.mean(jnp.square(x - m), ax, keepdims=True)
    xn = (x - m) / jnp.sqrt(v + eps)
    if g is None:
        return xn
    return xn * g + b


def _forward(x, k_w, k_b, q_w, q_b, v_w, v_b, kn_g, kn_b, qn_g, qn_b, vn_g,
             vn_b, klin_w, klin_b, qlin_w, qlin_b, alin_w, alin_b,
             l1_w, l1_b, l2_w, l2_b, fc1_w, fc1_b, fc2_w, fc2_b, fc3_w, fc3_b):
    b = x.shape[0]

    def proj(w, bias):
        t = x @ w.T + bias
        return t.reshape(b, N, H, D).transpose(0, 2, 1, 3)

    K = _ln(proj(k_w, k_b), kn_g, kn_b, 3)
    Q = _ln(proj(q_w, q_b), qn_g, qn_b, 3)
    V = _ln(proj(v_w, v_b), vn_g, vn_b, 3)
    A = jax.nn.elu((Q @ qlin_w.T + qlin_b) + (K @ klin_w.T + klin_b))
    A = A @ alin_w.T + alin_b
    A = jax.nn.softmax(A, axis=-1)
    E = jnp.einsum('bhfc,bhcd->bhfd', A, V)
    E = E.transpose(0, 2, 1, 3).reshape(b, N, H * D)
    E = jax.nn.relu(E @ l1_w.T + l1_b)
    E = _ln(E, None, None, 2)
    E = E.max(axis=1)
    y = jax.nn.elu(E @ l2_w.T + l2_b)
    a6 = jnp.tanh(y @ fc1_w.T + fc1_b)
    a7 = jnp.tanh(a6 @ fc2_w.T + fc2_b)
    return jax.nn.sigmoid(a7 @ fc3_w.T + fc3_b)


_pforward = jax.pmap(
    _forward,
    in_axes=(0,) + (None,) * len(WEIGHT_NAMES),
    devices=jax.devices()[:N_CORES],
)


def kernel(**inputs) -> np.ndarray:
    x = np.asarray(inputs["x"], dtype=np.float32).reshape(N_CORES, B_SHARD, N, F_IN)
    weights = [np.asarray(inputs[k], dtype=np.float32) for k in WEIGHT_NAMES]
    out = _pforward(x, *weights)
    return np.asarray(out).reshape(B, 1).astype(np.float32)
